# revision 37
# baseline (speedup 1.0000x reference)
"""Trainium2 Bass kernel for BasicQuantConv2d (sync-BN + HWGQ + gauss-quant + 3x3 conv).

Strategy (8 NeuronCores, data-parallel over batch):
  - Each core takes 4 of the 32 images: x shard [4, 128, 56, 56].
  - BN batch stats: per-core bn_stats/bn_aggr -> (mean, E[x^2])/8 payload,
    AllGather across the 8 cores + local 8-way sum (cheaper than AllReduce:
    the collective cost is dominated by fixed latency and AllReduce pays a
    1.875x multiplier on it), then per-channel scale/bias.
  - BN + HWGQ folds to ia = RNE_round(clip(x*s_c + b_c, 0, 3)) in {0..3};
    RNE rounding via the fp32 magic constant 1.5*2^23 (matches jnp.round).
  - gauss_quantize(w) == iw * (step/2) with iw in {-3,-1,1,3}; std(w) is
    computed on-device; weights transposed per-tap on the PE for the conv lhsT.
  - The 3x3 conv runs in fp8e4m3 (ia in {0..3}, iw in {-3,-1,1,3} are exact
    in fp8; PSUM accumulates fp32 => conv is EXACT integer arithmetic).
    5 PSUM groups per row-chunk: 3 vertical DoubleRow pairs (kh=0&1 per kw,
    pair-step 64B via the padded row width), 1 horizontal DoubleRow pair
    ((2,0)&(2,1), pair-step 1B), 1 single (2,2). Chunk-pairs drain early.
  - Engine split per image: ScalarE 3 BN-apply chunks + 2 pair-drains, Pool
    1 BN-apply chunk + 2 clip chunks (SBUF-only; GPSIMD cannot touch PSUM),
    DVE 2 clips + 4 rounds + 2 drains. PSUM drains are per chunk-pair
    (one instruction spanning both banks) scaled by alpha.
  - fp16 output: the conv result is alpha * (exact small integers); fp16
    adds <= half-ulp (~3.5e-4 relative) but halves the output DMA stream.
  - `_build(n_iters=K)` software-pipelines the K bodies: iteration k+1's
    x-load/bn-stats/payload/collective overlap iteration k's quantize+conv,
    so the collective's fixed latency leaves the steady-state cycle.

test.py measures per-iteration device time as (T(33)-T(1))/32 through the
~80ms axon RPC floor.
"""

import numpy as np

import concourse.bacc as bacc
import concourse.bass as bass
import concourse.tile as tile
from concourse import mybir
from concourse.masks import make_identity

N_CORES = 8
IMG = 4            # images per core
C = 128            # channels (= partitions)
HW = 56
S = HW * HW        # 3136 pixels per image
PR = 58            # padded rows
PCW = 64           # padded row width (interior at cols 2..57)
R = 8              # output rows per matmul tile
NT = HW // R       # 7 row-chunks per image
NFREE = R * HW     # 448 matmul free dim

HWGQ_STEP = 0.538
GAUSS = 0.996
BN_EPS = 1e-3
MAGIC = float(np.float32(1.5 * 2**23))
NW = 128 * 128 * 9          # weight element count

SA = 4 * 448   # first 32 rows of an image
SB = 3 * 448   # last 24 rows

# transposed-weight slot order: (kh=0,kw) & (kh=1,kw) adjacent for the
# vertical DoubleRow pairs; (2,0),(2,1) adjacent for the horizontal pair.
SLOT = {(0, 0): 0, (1, 0): 1, (0, 1): 2, (1, 1): 3,
        (0, 2): 4, (1, 2): 5, (2, 0): 6, (2, 1): 7, (2, 2): 8}

_CACHE = {}


class _P:
    """Pool/param bundle passed around emission helpers."""

    def __init__(self, nc, pools, params, shared):
        self.nc = nc
        (self.xp, self.apadp, self.wp, self.tmpp, self.outp, self.smallp,
         self.psump, self.psmallp, self.dramp) = pools
        (self.x_d, self.gamma_d, self.beta_d, self.w_d, self.y_d) = params
        self.shared = shared


def _alloc_state(P, k):
    fp32 = mybir.dt.float32
    st = {"k": k}
    st["xA"] = [P.xp.tile([C, SA], fp32, tag=f"xa{i}", name=f"xA{i}_{k}")
                for i in range(IMG)]
    st["xB"] = [P.xp.tile([C, SB], fp32, tag=f"xb{i}", name=f"xB{i}_{k}")
                for i in range(IMG)]
    st["w_sb"] = P.wp.tile([C, 128 * 9], fp32, tag="wsb", name=f"wsb_{k}")
    st["gb"] = P.smallp.tile([C, 2], fp32, tag="gb", name=f"gb_{k}")
    st["stats"] = P.smallp.tile([C, IMG * 7, 6], fp32, tag="stats",
                                name=f"stats_{k}")
    st["mv"] = P.smallp.tile([C, 2], fp32, tag="mv", name=f"mv_{k}")
    st["pay8"] = P.smallp.tile([C, 2], fp32, tag="pay8", name=f"pay8_{k}")
    st["cc_in"] = P.dramp.tile([C, 2], fp32, tag="ccin", name=f"ccin_{k}")
    st["cc_out"] = P.dramp.tile([N_CORES, C, 2], fp32, tag="ccout",
                                name=f"ccout_{k}")
    return st


def _emit_x_dmas(P, st, i):
    """x DMAs for image i of state st (img 3 split finer for the stats tail)."""
    nc = P.nc
    if i < IMG - 1:
        nc.sync.dma_start(out=st["xA"][i][:], in_=P.x_d.ap()[i][:, 0:SA])
        nc.sync.dma_start(out=st["xB"][i][:], in_=P.x_d.ap()[i][:, SA:S])
    else:
        for (lo, hi) in ((0, 896), (896, 1792)):
            nc.sync.dma_start(out=st["xA"][i][:, lo:hi],
                              in_=P.x_d.ap()[i][:, lo:hi])
        nc.sync.dma_start(out=st["xB"][i][:, 0:896],
                          in_=P.x_d.ap()[i][:, SA:SA + 896])
        nc.sync.dma_start(out=st["xB"][i][:, 896:SB],
                          in_=P.x_d.ap()[i][:, SA + 896:S])


def _emit_stats(P, st, i):
    nc = P.nc
    ga = st["xA"][i][:].rearrange("p (g f) -> p g f", g=4)
    gb_ = st["xB"][i][:].rearrange("p (g f) -> p g f", g=3)
    for g in range(7):
        if g < 4:
            nc.vector.bn_stats(out=st["stats"][:, i * 7 + g, :], in_=ga[:, g, :])
        else:
            nc.vector.bn_stats(out=st["stats"][:, i * 7 + g, :],
                               in_=gb_[:, g - 4, :])


def _emit_payload(P, st):
    """bn_aggr + payload build + DMA to DRAM."""
    nc = P.nc
    fp32 = mybir.dt.float32
    mv, pay8 = st["mv"], st["pay8"]
    nc.vector.bn_aggr(out=mv[:], in_=st["stats"][:])
    m2 = P.smallp.tile([C, 1], fp32, tag="m2", name=f"m2_{st['k']}")
    ex2 = P.smallp.tile([C, 1], fp32, tag="ex2", name=f"ex2_{st['k']}")
    nc.vector.tensor_mul(m2[:], mv[:, 0:1], mv[:, 0:1])
    nc.vector.tensor_add(ex2[:], mv[:, 1:2], m2[:])
    nc.vector.tensor_scalar_mul(pay8[:, 0:1], mv[:, 0:1], 1.0 / N_CORES)
    nc.vector.tensor_scalar_mul(pay8[:, 1:2], ex2[:], 1.0 / N_CORES)
    nc.sync.dma_start(out=st["cc_in"][:], in_=pay8[:])


def _emit_collective(P, st, ablate):
    nc = P.nc
    OP = mybir.AluOpType
    if "noar" in ablate:
        for r in range(N_CORES):
            nc.sync.dma_start(out=st["cc_out"][r], in_=st["cc_in"][:])
    else:
        nc.gpsimd.collective_compute(
            "AllGather",
            OP.bypass,
            replica_groups=[list(range(N_CORES))],
            ins=[st["cc_in"].opt()],
            outs=[st["cc_out"].opt()],
        )


def _emit_gb_dma(P, st):
    nc = P.nc
    gamma_ap = P.gamma_d.ap().rearrange("(p one) -> p one", one=1)
    beta_ap = P.beta_d.ap().rearrange("(p one) -> p one", one=1)
    nc.sync.dma_start(out=st["gb"][:, 0:1], in_=gamma_ap)
    nc.sync.dma_start(out=st["gb"][:, 1:2], in_=beta_ap)


def _emit_weight_path(P, st):
    """Transpose + global-std + quantize of st's weights.

    Runs on phase-1/idle slack: PE transposes + ScalarE copies/accums, DVE
    std chain, Pool fp8 quantize. Produces st["wq"] (fp8 lhsT slots) and
    st["alpha"] (output scale).
    """
    nc = P.nc
    fp32 = mybir.dt.float32
    fp8 = mybir.dt.float8e4
    AF = mybir.ActivationFunctionType
    OP = mybir.AluOpType
    k = st["k"]
    sh = P.shared

    wT = P.wp.tile([C, 9, 128], fp32, tag="wT", name=f"wT_{k}")
    w3 = st["w_sb"][:].rearrange("p (ci t) -> p ci t", t=9)
    for t in range(9):
        kh, kw = divmod(t, 3)
        pt = P.psmallp.tile([C, 128], fp32, tag="psm", name=f"pt_{k}_{t}")
        nc.tensor.transpose(pt[:], w3[:, :, t], sh["ident"][:])
        nc.scalar.copy(out=wT[:, SLOT[(kh, kw)], :], in_=pt[:])

    # global sum / sumsq via ScalarE accum_out + ones-matmul broadcast;
    # the activation main outputs are scratch (routed into uw, overwritten
    # later by the quantize chain).
    uw = P.wp.tile([C, 9, 128], fp32, tag="uw", name=f"uw_{k}")
    uwf = uw[:].rearrange("p a b -> p (a b)")
    rsums = P.smallp.tile([C, 2], fp32, tag="rsums", name=f"rsums_{k}")
    nc.scalar.activation(out=uwf, in_=st["w_sb"][:], func=AF.Identity,
                         accum_out=rsums[:, 0:1])
    nc.scalar.activation(out=uwf, in_=st["w_sb"][:], func=AF.Square,
                         accum_out=rsums[:, 1:2])
    pg = P.psmallp.tile([C, 128], fp32, tag="psm", name=f"pg_{k}")
    nc.tensor.matmul(pg[:, 0:2], lhsT=sh["ones"][:], rhs=rsums[:],
                     start=True, stop=True)
    gs = P.smallp.tile([C, 2], fp32, tag="gs", name=f"gs_{k}")
    nc.vector.tensor_copy(gs[:], pg[:, 0:2])

    wmean = P.smallp.tile([C, 1], fp32, tag="wmean", name=f"wmean_{k}")
    wvar = P.smallp.tile([C, 1], fp32, tag="wvar", name=f"wvar_{k}")
    nc.vector.tensor_scalar_mul(wmean[:], gs[:, 0:1], 1.0 / NW)
    nc.vector.tensor_scalar_mul(wvar[:], gs[:, 1:2], 1.0 / NW)
    wm2 = P.smallp.tile([C, 1], fp32, tag="wm2", name=f"wm2_{k}")
    nc.vector.tensor_mul(wm2[:], wmean[:], wmean[:])
    nc.vector.tensor_sub(wvar[:], wvar[:], wm2[:])

    rw = P.smallp.tile([C, 1], fp32, tag="rw", name=f"rw_{k}")
    nc.scalar.activation(out=rw[:], in_=wvar[:], func=AF.Sqrt)
    nc.vector.reciprocal(out=rw[:], in_=rw[:])
    tN = P.smallp.tile([C, 1], fp32, tag="tN", name=f"tN_{k}")
    for _ in range(2):
        nc.vector.tensor_mul(tN[:], rw[:], rw[:])
        nc.vector.tensor_mul(tN[:], wvar[:], tN[:])
        nc.vector.tensor_scalar(tN[:], tN[:], -0.5, 1.5, OP.mult, OP.add)
        nc.vector.tensor_mul(rw[:], rw[:], tN[:])

    inv_step = P.smallp.tile([C, 1], fp32, tag="invs", name=f"invs_{k}")
    nc.vector.tensor_scalar_mul(inv_step[:], rw[:], 1.0 / GAUSS)
    alpha = P.smallp.tile([C, 1], fp32, tag="alpha", name=f"alpha_{k}")
    nc.vector.tensor_mul(alpha[:], wvar[:], rw[:])
    nc.vector.tensor_scalar_mul(alpha[:], alpha[:], HWGQ_STEP * GAUSS / 2.0)

    # quantize transposed weights -> iw in {-3,-1,1,3} (fp8) on Pool
    wq = P.wp.tile([C, 9, 128], fp8, tag="wq", name=f"wq_{k}")
    nc.gpsimd.tensor_scalar(uw[:], wT[:], inv_step[:], 0.5, OP.mult, OP.add)
    nc.gpsimd.tensor_scalar(uw[:], uw[:], MAGIC, MAGIC, OP.add, OP.subtract)
    nc.gpsimd.tensor_scalar(uw[:], uw[:], 2.0, -1.0, OP.mult, OP.add)
    nc.gpsimd.tensor_scalar(wq[:], uw[:], 3.0, -3.0, OP.min, OP.max)
    st["wq"] = wq
    st["alpha"] = alpha


def _emit_block(P, st, ld, ablate):
    """Phase 3 of `st` (gather -> quantize -> conv -> drain -> y) interleaved
    with the loads/stats/payload/collective of `ld` (may be None)."""
    nc = P.nc
    fp32 = mybir.dt.float32
    fp16 = mybir.dt.float16
    AF = mybir.ActivationFunctionType
    OP = mybir.AluOpType
    k = st["k"]
    sh = P.shared

    # ---------------- gather hop + 8-way sum ----------------
    g16 = P.smallp.tile([C, 16], fp32, tag="g16", name=f"g16_{k}")
    cc_ap = st["cc_out"].opt()
    nc.sync.dma_start(out=g16[:], in_=bass.AP(
        tensor=cc_ap.tensor, offset=cc_ap.offset,
        ap=[[2, C], [2 * C, N_CORES], [1, 2]]))
    t8 = P.smallp.tile([C, 8], fp32, tag="t8", name=f"t8_{k}")
    t4 = P.smallp.tile([C, 4], fp32, tag="t4", name=f"t4_{k}")
    g2 = P.smallp.tile([C, 2], fp32, tag="g2", name=f"g2_{k}")
    nc.vector.tensor_add(t8[:], g16[:, 0:8], g16[:, 8:16])
    nc.vector.tensor_add(t4[:], t8[:, 0:4], t8[:, 4:8])
    nc.vector.tensor_add(g2[:], t4[:, 0:2], t4[:, 2:4])

    # ---------------- BN epilogue: s = gamma*rsqrt/0.538, b = ... ----------
    vge = P.smallp.tile([C, 1], fp32, tag="vge", name=f"vge_{k}")
    gm2 = P.smallp.tile([C, 1], fp32, tag="gm2", name=f"gm2_{k}")
    nc.vector.tensor_mul(gm2[:], g2[:, 0:1], g2[:, 0:1])
    nc.vector.tensor_sub(vge[:], g2[:, 1:2], gm2[:])
    nc.vector.tensor_scalar_add(vge[:], vge[:], BN_EPS)
    rx = P.smallp.tile([C, 1], fp32, tag="rx", name=f"rx_{k}")
    nc.scalar.activation(out=rx[:], in_=vge[:], func=AF.Sqrt)
    nc.vector.reciprocal(out=rx[:], in_=rx[:])
    tX = P.smallp.tile([C, 1], fp32, tag="tX", name=f"tX_{k}")
    for _ in range(2):
        nc.vector.tensor_mul(tX[:], rx[:], rx[:])
        nc.vector.tensor_mul(tX[:], vge[:], tX[:])
        nc.vector.tensor_scalar(tX[:], tX[:], -0.5, 1.5, OP.mult, OP.add)
        nc.vector.tensor_mul(rx[:], rx[:], tX[:])

    s_q = P.smallp.tile([C, 1], fp32, tag="s_q", name=f"s_q_{k}")
    b_q = P.smallp.tile([C, 1], fp32, tag="b_q", name=f"b_q_{k}")
    ta = P.smallp.tile([C, 1], fp32, tag="ta", name=f"ta_{k}")
    tb = P.smallp.tile([C, 1], fp32, tag="tb", name=f"tb_{k}")
    nc.vector.tensor_mul(ta[:], st["gb"][:, 0:1], rx[:])
    nc.vector.tensor_scalar_mul(s_q[:], ta[:], 1.0 / HWGQ_STEP)
    nc.vector.tensor_mul(tb[:], g2[:, 0:1], ta[:])
    nc.vector.tensor_sub(tb[:], st["gb"][:, 1:2], tb[:])
    nc.vector.tensor_scalar_mul(b_q[:], tb[:], 1.0 / HWGQ_STEP)

    # ---------------- PE warmups (p-state ramp into the conv burst) --------
    if "nowarm" not in ablate:
        wu = P.smallp.tile([C, 128], fp32, tag="wu", name=f"wu_{k}")
        nc.vector.tensor_scalar_mul(wu[:], st["xA"][0][:, 0:128], g2[:, 0:1])
        ps_warm = P.psmallp.tile([C, 128], fp32, tag="psm", name=f"psw_{k}")
        for _ in range(6):
            nc.tensor.matmul(ps_warm[:], lhsT=sh["ones"][:], rhs=wu[:],
                             start=True, stop=True)

    # ---------------- per-image quantize + conv + drain + y ----------------
    wq, alpha = st["wq"], st["alpha"]
    a_t = sh["a_t"]
    out_sbs = [None] * IMG
    pss = [None] * IMG

    def quantize(i):
        u_sb = P.tmpp.tile([C, S], fp32, tag="u", name=f"u_{k}_{i}")
        # BN-apply chunks: ScalarE takes 3, Pool takes (1792:2688)
        nc.scalar.activation(out=u_sb[:, 0:896], in_=st["xA"][i][:, 0:896],
                             func=AF.Identity, bias=b_q[:], scale=s_q[:])
        nc.scalar.activation(out=u_sb[:, 896:SA], in_=st["xA"][i][:, 896:SA],
                             func=AF.Identity, bias=b_q[:], scale=s_q[:])
        nc.gpsimd.tensor_scalar(u_sb[:, SA:2688], st["xB"][i][:, 0:896],
                                s_q[:], b_q[:], OP.mult, OP.add)
        nc.scalar.activation(out=u_sb[:, 2688:S], in_=st["xB"][i][:, 896:SB],
                             func=AF.Identity, bias=b_q[:], scale=s_q[:])
        # clip (in place): DVE rows (0,16),(48,56); Pool rows (16,32),(32,48)
        for (r0, r1, eng) in ((0, 16, "D"), (16, 32, "P"),
                              (32, 48, "P"), (48, 56, "D")):
            lo, hi = r0 * HW, r1 * HW
            e = nc.vector if eng == "D" else nc.gpsimd
            e.tensor_scalar(u_sb[:, lo:hi], u_sb[:, lo:hi], 3.0, 0.0,
                            OP.min, OP.max)
        # round -> fp8 interior (DVE)
        for (r0, r1) in ((0, 16), (16, 32), (32, 48), (48, 56)):
            lo, hi = r0 * HW, r1 * HW
            nc.vector.tensor_scalar(a_t[i][:, r0 + 1:r1 + 1, 2:58],
                                    u_sb[:, lo:hi].rearrange(
                                        "p (h w) -> p h w", h=r1 - r0),
                                    MAGIC, MAGIC, OP.add, OP.subtract)
        out_sbs[i] = P.outp.tile([C, S], fp16, tag="o", name=f"o_{k}_{i}")
        pss[i] = [P.psump.tile([C, NFREE], fp32, tag=f"ps{c}",
                               name=f"ps_{k}_{i}_{c}") for c in range(NT)]

    def conv_chunk(i, cix, g):
        base = a_t[i][:]
        ps = pss[i]
        h0 = cix * R
        if g < 3:
            kw = g
            rhs = bass.AP(
                tensor=base.tensor,
                offset=base.offset + (h0 + 0) * PCW + (kw + 1),
                ap=[base.ap[0], [PCW, 2], [PCW, R], [1, HW]],
            )
            nc.tensor.matmul(ps[cix][:], lhsT=wq[:, 2 * kw: 2 * kw + 2, :],
                             rhs=rhs, start=(g == 0), stop=False,
                             perf_mode=mybir.MatmulPerfMode.DoubleRow)
        elif g == 3:
            # horizontal pair (kh=2, kw=0)&(kh=2, kw=1): slots 6,7
            rhs = bass.AP(
                tensor=base.tensor,
                offset=base.offset + (h0 + 2) * PCW + 1,
                ap=[base.ap[0], [1, 2], [PCW, R], [1, HW]],
            )
            nc.tensor.matmul(ps[cix][:], lhsT=wq[:, 6:8, :],
                             rhs=rhs, start=False, stop=False,
                             perf_mode=mybir.MatmulPerfMode.DoubleRow)
        else:
            rhs = a_t[i][:, h0 + 2: h0 + 2 + R, 3: 3 + HW]
            nc.tensor.matmul(ps[cix][:], lhsT=wq[:, 8, :], rhs=rhs,
                             start=False, stop=True)

    def drain_pair(i, pair, eng):
        ps = pss[i]
        for cix in pair:
            lo = cix * NFREE
            dst = out_sbs[i][:, lo:lo + NFREE]
            if eng == "S":
                nc.scalar.activation(out=dst, in_=ps[cix][:], func=AF.Identity,
                                     scale=alpha[:])
            else:
                nc.vector.tensor_scalar_mul(dst, ps[cix][:], alpha[:])

    PAIRS = ((0, 1), (2, 3), (4, 5), (6,))
    PAIR_ENG = {0: "S", 1: "S", 2: "S", 3: "D"}

    def conv_image(i):
        for pi, pair in enumerate(PAIRS):
            for g in range(5):
                for cix in pair:
                    conv_chunk(i, cix, g)
            drain_pair(i, pair, PAIR_ENG[pi])
            lo = pair[0] * NFREE
            hi = (pair[-1] + 1) * NFREE
            nc.sync.dma_start(out=P.y_d.ap()[i][:, lo:hi],
                              in_=out_sbs[i][:, lo:hi])

    # stagger: quantize(i) stays one image ahead of conv(i-1) so the
    # ScalarE/DVE queues never block the next image's act/clip behind the
    # previous image's PSUM drains.
    for i in range(IMG):
        if ld is not None:
            _emit_x_dmas(P, ld, i)
            if i == 1:
                _emit_gb_dma(P, ld)
            if i == IMG - 1:
                nc.sync.dma_start(out=ld["w_sb"][:], in_=P.w_d.ap())
        quantize(i)
        if ld is not None:
            _emit_stats(P, ld, i)
            if i == IMG - 1:
                _emit_payload(P, ld)
        if i >= 1:
            conv_image(i - 1)
    conv_image(IMG - 1)

    if ld is not None:
        # Pool order: collective first (it only waits on the payload DMA),
        # then the weight-quantize ops which depend on the late w DMA.
        _emit_collective(P, ld, ablate)
        _emit_weight_path(P, ld)


def _emit_cold_start(P, st, ablate):
    """First iteration's phase 1: loads + stats + payload + collective +
    weight path, plus one-time shared init (identity, ones, a_t borders)."""
    nc = P.nc
    fp32 = mybir.dt.float32
    fp8 = mybir.dt.float8e4
    sh = P.shared

    for i in range(IMG):
        _emit_x_dmas(P, st, i)
        _emit_stats(P, st, i)
    _emit_gb_dma(P, st)
    nc.sync.dma_start(out=st["w_sb"][:], in_=P.w_d.ap())
    _emit_payload(P, st)

    # one-time shared init
    sh["ident"] = P.smallp.tile([C, 128], fp32, tag="ident", name="ident")
    make_identity(nc, sh["ident"][:])
    sh["ones"] = P.smallp.tile([C, 128], fp32, tag="ones", name="ones")
    nc.vector.memset(sh["ones"][:], 1.0)
    sh["a_t"] = [P.apadp.tile([C, PR, PCW], fp8, tag=f"a{i}", name=f"a_t{i}")
                 for i in range(IMG)]
    for i in range(IMG):
        nc.gpsimd.memset(sh["a_t"][i][:, 0, :], 0.0)
        nc.gpsimd.memset(sh["a_t"][i][:, 57, :], 0.0)
        nc.gpsimd.memset(sh["a_t"][i][:, 1:57, 0:2], 0.0)
        nc.gpsimd.memset(sh["a_t"][i][:, 1:57, 58:64], 0.0)

    _emit_collective(P, st, ablate)
    _emit_weight_path(P, st)


def _build(n_iters=1, ablate=()):
    fp32 = mybir.dt.float32

    nc = bacc.Bacc("TRN2", target_bir_lowering=False, debug=False,
                   num_devices=N_CORES)

    x_d = nc.declare_dram_parameter("x", [IMG, C, S], fp32, isOutput=False)
    gamma_d = nc.declare_dram_parameter("gamma", [C], fp32, isOutput=False)
    beta_d = nc.declare_dram_parameter("beta", [C], fp32, isOutput=False)
    w_d = nc.declare_dram_parameter("weight", [C, 128 * 9], fp32, isOutput=False)
    # fp16 output: halves the y DMA stream; upconverted on host.
    y_d = nc.declare_dram_parameter("y", [IMG, C, S], mybir.dt.float16,
                                    isOutput=True)
    params = (x_d, gamma_d, beta_d, w_d, y_d)

    with tile.TileContext(nc) as tc:
        with (
            tc.tile_pool(name="xp", bufs=2) as xp,
            tc.tile_pool(name="apad", bufs=1) as apadp,
            tc.tile_pool(name="wp", bufs=1) as wp,
            tc.tile_pool(name="tmp", bufs=2) as tmpp,
            tc.tile_pool(name="outp", bufs=4) as outp,
            tc.tile_pool(name="small", bufs=1) as smallp,
            tc.tile_pool(name="psum", bufs=1, space="PSUM") as psump,
            tc.tile_pool(name="psmall", bufs=1, space="PSUM") as psmallp,
            tc.tile_pool(name="dram", bufs=2, space="DRAM") as dramp,
        ):
            pools = (xp, apadp, wp, tmpp, outp, smallp, psump, psmallp, dramp)
            P = _P(nc, pools, params, {})
            st = _alloc_state(P, 0)
            _emit_cold_start(P, st, ablate)
            for k in range(1, n_iters):
                ld = _alloc_state(P, k)
                _emit_block(P, st, ld, ablate)
                st = ld
            _emit_block(P, st, None, ablate)

    nc.finalize()
    return nc


def _get_nc(n_iters=1):
    key = ("nc", n_iters)
    if key not in _CACHE:
        _CACHE[key] = _build(n_iters)
    return _CACHE[key]


def make_in_maps(x, gamma, beta, weight):
    x = np.ascontiguousarray(np.asarray(x, np.float32)).reshape(N_CORES, IMG, C, S)
    w = np.ascontiguousarray(np.asarray(weight, np.float32)).reshape(C, 128 * 9)
    gamma = np.ascontiguousarray(np.asarray(gamma, np.float32))
    beta = np.ascontiguousarray(np.asarray(beta, np.float32))
    return [
        {"x": x[c], "gamma": gamma, "beta": beta, "weight": w}
        for c in range(N_CORES)
    ]


def kernel(x, gamma, beta, weight):
    import os
    from concourse.bass_utils import run_bass_kernel_spmd

    nc = _get_nc()
    in_maps = make_in_maps(x, gamma, beta, weight)
    core_ids = list(range(N_CORES))
    try:
        res = run_bass_kernel_spmd(nc, in_maps, core_ids)
    except ModuleNotFoundError:
        # BASS_TRACE set but no NTFF profile hook in this container
        os.environ["BASS_NEVER_TRACE"] = "1"
        res = run_bass_kernel_spmd(nc, in_maps, core_ids)
    out = np.stack([res.results[c]["y"] for c in range(N_CORES)], axis=0)
    return out.reshape(32, C, HW, HW).astype(np.float32)


# revision 40
# speedup vs baseline: 2.7210x; 2.7210x over previous
"""Trainium2 Bass kernel for BasicQuantConv2d (sync-BN + HWGQ + gauss-quant + 3x3 conv).

Strategy (8 NeuronCores, data-parallel over batch):
  - Each core takes 4 of the 32 images: x shard [4, 128, 56, 56].
  - BN batch stats: per-core bn_stats/bn_aggr -> (mean, E[x^2])/8 payload,
    AllGather across the 8 cores + local 8-way sum (cheaper than AllReduce:
    the collective cost is dominated by fixed latency and AllReduce pays a
    1.875x multiplier on it), then per-channel scale/bias.
  - BN + HWGQ folds to ia = RNE_round(clip(x*s_c + b_c, 0, 3)) in {0..3};
    RNE rounding via the fp32 magic constant 1.5*2^23 (matches jnp.round).
  - gauss_quantize(w) == iw * (step/2) with iw in {-3,-1,1,3}; std(w) is
    computed on-device; weights transposed per-tap on the PE for the conv lhsT.
  - The 3x3 conv runs in fp8e4m3 (ia in {0..3}, iw in {-3,-1,1,3} are exact
    in fp8; PSUM accumulates fp32 => conv is EXACT integer arithmetic).
    5 PSUM groups per row-chunk: 3 vertical DoubleRow pairs (kh=0&1 per kw,
    pair-step 64B via the padded row width), 1 horizontal DoubleRow pair
    ((2,0)&(2,1), pair-step 1B), 1 single (2,2). Chunk-pairs drain early.
  - Engine split per image: ScalarE 3 BN-apply chunks + 2 pair-drains, Pool
    1 BN-apply chunk + 2 clip chunks (SBUF-only; GPSIMD cannot touch PSUM),
    DVE 2 clips + 4 rounds + 2 drains. PSUM drains are per chunk-pair
    (one instruction spanning both banks) scaled by alpha.
  - fp16 output: the conv result is alpha * (exact small integers); fp16
    adds <= half-ulp (~3.5e-4 relative) but halves the output DMA stream.
  - `_build(n_iters=K)` software-pipelines the K bodies: iteration k+1's
    x-load/bn-stats/payload/collective overlap iteration k's quantize+conv,
    so the collective's fixed latency leaves the steady-state cycle.

test.py measures per-iteration device time as (T(33)-T(1))/32 through the
~80ms axon RPC floor.
"""

import numpy as np

import concourse.bacc as bacc
import concourse.bass as bass
import concourse.tile as tile
from concourse import mybir
from concourse.masks import make_identity

N_CORES = 8
IMG = 4            # images per core
C = 128            # channels (= partitions)
HW = 56
S = HW * HW        # 3136 pixels per image
PR = 58            # padded rows
PCW = 64           # padded row width (interior at cols 2..57)
R = 8              # output rows per matmul tile
NT = HW // R       # 7 row-chunks per image
NFREE = R * HW     # 448 matmul free dim

HWGQ_STEP = 0.538
GAUSS = 0.996
BN_EPS = 1e-3
MAGIC = float(np.float32(1.5 * 2**23))
NW = 128 * 128 * 9          # weight element count

SA = 4 * 448   # first 32 rows of an image
SB = 3 * 448   # last 24 rows

# transposed-weight slot order: (kh=0,kw) & (kh=1,kw) adjacent for the
# vertical DoubleRow pairs; (2,0),(2,1) adjacent for the horizontal pair.
SLOT = {(0, 0): 0, (1, 0): 1, (0, 1): 2, (1, 1): 3,
        (0, 2): 4, (1, 2): 5, (2, 0): 6, (2, 1): 7, (2, 2): 8}

_CACHE = {}


class _P:
    """Pool/param bundle passed around emission helpers."""

    def __init__(self, nc, pools, params, shared):
        self.nc = nc
        (self.xp, self.apadp, self.wp, self.tmpp, self.outp, self.smallp,
         self.psump, self.psmallp, self.dramp) = pools
        (self.x_d, self.gamma_d, self.beta_d, self.w_d, self.y_d) = params
        self.shared = shared


def _alloc_state(P, k):
    fp32 = mybir.dt.float32
    st = {"k": k}
    st["xA"] = [P.xp.tile([C, SA], fp32, tag=f"xa{i}", name=f"xA{i}_{k}")
                for i in range(IMG)]
    st["xB"] = [P.xp.tile([C, SB], fp32, tag=f"xb{i}", name=f"xB{i}_{k}")
                for i in range(IMG)]
    st["w_sb"] = P.wp.tile([C, 128 * 9], fp32, tag="wsb", name=f"wsb_{k}")
    st["gb"] = P.smallp.tile([C, 2], fp32, tag="gb", name=f"gb_{k}")
    st["stats"] = P.smallp.tile([C, IMG * 7, 6], fp32, tag="stats",
                                name=f"stats_{k}")
    st["mv"] = P.smallp.tile([C, 2], fp32, tag="mv", name=f"mv_{k}")
    st["pay8"] = P.smallp.tile([C, 2], fp32, tag="pay8", name=f"pay8_{k}")
    st["cc_in"] = P.dramp.tile([C, 2], fp32, tag="ccin", name=f"ccin_{k}")
    st["cc_out"] = P.dramp.tile([N_CORES, C, 2], fp32, tag="ccout",
                                name=f"ccout_{k}")
    return st


def _emit_x_dmas(P, st, i):
    """x DMAs for image i of state st (img 3 split finer for the stats tail)."""
    nc = P.nc
    if i < IMG - 1:
        nc.sync.dma_start(out=st["xA"][i][:], in_=P.x_d.ap()[i][:, 0:SA])
        nc.sync.dma_start(out=st["xB"][i][:], in_=P.x_d.ap()[i][:, SA:S])
    else:
        for (lo, hi) in ((0, 896), (896, 1792)):
            nc.sync.dma_start(out=st["xA"][i][:, lo:hi],
                              in_=P.x_d.ap()[i][:, lo:hi])
        nc.sync.dma_start(out=st["xB"][i][:, 0:896],
                          in_=P.x_d.ap()[i][:, SA:SA + 896])
        nc.sync.dma_start(out=st["xB"][i][:, 896:SB],
                          in_=P.x_d.ap()[i][:, SA + 896:S])


def _emit_stats(P, st, i):
    nc = P.nc
    ga = st["xA"][i][:].rearrange("p (g f) -> p g f", g=4)
    gb_ = st["xB"][i][:].rearrange("p (g f) -> p g f", g=3)
    for g in range(7):
        if g < 4:
            nc.vector.bn_stats(out=st["stats"][:, i * 7 + g, :], in_=ga[:, g, :])
        else:
            nc.vector.bn_stats(out=st["stats"][:, i * 7 + g, :],
                               in_=gb_[:, g - 4, :])


def _emit_payload(P, st):
    """bn_aggr + payload build + DMA to DRAM."""
    nc = P.nc
    fp32 = mybir.dt.float32
    mv, pay8 = st["mv"], st["pay8"]
    nc.vector.bn_aggr(out=mv[:], in_=st["stats"][:])
    m2 = P.smallp.tile([C, 1], fp32, tag="m2", name=f"m2_{st['k']}")
    ex2 = P.smallp.tile([C, 1], fp32, tag="ex2", name=f"ex2_{st['k']}")
    nc.vector.tensor_mul(m2[:], mv[:, 0:1], mv[:, 0:1])
    nc.vector.tensor_add(ex2[:], mv[:, 1:2], m2[:])
    nc.vector.tensor_scalar_mul(pay8[:, 0:1], mv[:, 0:1], 1.0 / N_CORES)
    nc.vector.tensor_scalar_mul(pay8[:, 1:2], ex2[:], 1.0 / N_CORES)
    nc.sync.dma_start(out=st["cc_in"][:], in_=pay8[:])


def _emit_collective(P, st, ablate):
    nc = P.nc
    OP = mybir.AluOpType
    if "noar" in ablate:
        for r in range(N_CORES):
            nc.sync.dma_start(out=st["cc_out"][r], in_=st["cc_in"][:])
    else:
        nc.gpsimd.collective_compute(
            "AllGather",
            OP.bypass,
            replica_groups=[list(range(N_CORES))],
            ins=[st["cc_in"].opt()],
            outs=[st["cc_out"].opt()],
        )


def _emit_gb_dma(P, st):
    nc = P.nc
    gamma_ap = P.gamma_d.ap().rearrange("(p one) -> p one", one=1)
    beta_ap = P.beta_d.ap().rearrange("(p one) -> p one", one=1)
    nc.sync.dma_start(out=st["gb"][:, 0:1], in_=gamma_ap)
    nc.sync.dma_start(out=st["gb"][:, 1:2], in_=beta_ap)


def _emit_weight_path(P, st):
    """Transpose + global-std + quantize of st's weights.

    Runs on phase-1/idle slack: PE transposes + ScalarE copies/accums, DVE
    std chain, Pool fp8 quantize. Produces st["wq"] (fp8 lhsT slots) and
    st["alpha"] (output scale).
    """
    nc = P.nc
    fp32 = mybir.dt.float32
    fp8 = mybir.dt.float8e4
    AF = mybir.ActivationFunctionType
    OP = mybir.AluOpType
    k = st["k"]
    sh = P.shared

    wT = P.wp.tile([C, 9, 128], fp32, tag="wT", name=f"wT_{k}")
    w3 = st["w_sb"][:].rearrange("p (ci t) -> p ci t", t=9)
    # batch 3 transposes per PSUM bank, one wide copy each: the transposes
    # write disjoint ranges so they run back-to-back, and the copies are 3
    # wide instructions instead of 9 narrow ones.
    TAPORD = sorted(range(9), key=lambda t: SLOT[divmod(t, 3)])
    for b in range(3):
        pt = P.psmallp.tile([C, 3, 128], fp32, tag="psm", name=f"pt_{k}_{b}")
        for j in range(3):
            t = TAPORD[b * 3 + j]
            nc.tensor.transpose(pt[:, j, :], w3[:, :, t], sh["ident"][:])
        s0 = SLOT[divmod(TAPORD[b * 3], 3)]
        nc.scalar.copy(out=wT[:, s0:s0 + 3, :], in_=pt[:])

    # global sum / sumsq via ScalarE accum_out + ones-matmul broadcast;
    # the activation main outputs are scratch (routed into uw, overwritten
    # later by the quantize chain).
    uw = P.wp.tile([C, 9, 128], fp32, tag="uw", name=f"uw_{k}")
    uwf = uw[:].rearrange("p a b -> p (a b)")
    rsums = P.smallp.tile([C, 2], fp32, tag="rsums", name=f"rsums_{k}")
    nc.scalar.activation(out=uwf, in_=st["w_sb"][:], func=AF.Identity,
                         accum_out=rsums[:, 0:1])
    nc.scalar.activation(out=uwf, in_=st["w_sb"][:], func=AF.Square,
                         accum_out=rsums[:, 1:2])
    pg = P.psmallp.tile([C, 128], fp32, tag="psm", name=f"pg_{k}")
    nc.tensor.matmul(pg[:, 0:2], lhsT=sh["ones"][:], rhs=rsums[:],
                     start=True, stop=True)
    gs = P.smallp.tile([C, 2], fp32, tag="gs", name=f"gs_{k}")
    nc.vector.tensor_copy(gs[:], pg[:, 0:2])

    wmean = P.smallp.tile([C, 1], fp32, tag="wmean", name=f"wmean_{k}")
    wvar = P.smallp.tile([C, 1], fp32, tag="wvar", name=f"wvar_{k}")
    nc.vector.tensor_scalar_mul(wmean[:], gs[:, 0:1], 1.0 / NW)
    nc.vector.tensor_scalar_mul(wvar[:], gs[:, 1:2], 1.0 / NW)
    wm2 = P.smallp.tile([C, 1], fp32, tag="wm2", name=f"wm2_{k}")
    nc.vector.tensor_mul(wm2[:], wmean[:], wmean[:])
    nc.vector.tensor_sub(wvar[:], wvar[:], wm2[:])

    rw = P.smallp.tile([C, 1], fp32, tag="rw", name=f"rw_{k}")
    nc.scalar.activation(out=rw[:], in_=wvar[:], func=AF.Sqrt)
    nc.vector.reciprocal(out=rw[:], in_=rw[:])
    tN = P.smallp.tile([C, 1], fp32, tag="tN", name=f"tN_{k}")
    for _ in range(2):
        nc.vector.tensor_mul(tN[:], rw[:], rw[:])
        nc.vector.tensor_mul(tN[:], wvar[:], tN[:])
        nc.vector.tensor_scalar(tN[:], tN[:], -0.5, 1.5, OP.mult, OP.add)
        nc.vector.tensor_mul(rw[:], rw[:], tN[:])

    inv_step = P.smallp.tile([C, 1], fp32, tag="invs", name=f"invs_{k}")
    nc.vector.tensor_scalar_mul(inv_step[:], rw[:], 1.0 / GAUSS)
    alpha = P.smallp.tile([C, 1], fp32, tag="alpha", name=f"alpha_{k}")
    nc.vector.tensor_mul(alpha[:], wvar[:], rw[:])
    nc.vector.tensor_scalar_mul(alpha[:], alpha[:], HWGQ_STEP * GAUSS / 2.0)

    # quantize transposed weights -> iw in {-3,-1,1,3} (fp8) on Pool
    wq = P.wp.tile([C, 9, 128], fp8, tag="wq", name=f"wq_{k}")
    nc.gpsimd.tensor_scalar(uw[:], wT[:], inv_step[:], 0.5, OP.mult, OP.add)
    nc.gpsimd.tensor_scalar(uw[:], uw[:], MAGIC, MAGIC, OP.add, OP.subtract)
    nc.gpsimd.tensor_scalar(uw[:], uw[:], 2.0, -1.0, OP.mult, OP.add)
    nc.gpsimd.tensor_scalar(wq[:], uw[:], 3.0, -3.0, OP.min, OP.max)
    st["wq"] = wq
    st["alpha"] = alpha


def _emit_block(P, st, ld, ablate):
    """Phase 3 of `st` (gather -> quantize -> conv -> drain -> y) interleaved
    with the loads/stats/payload/collective of `ld` (may be None)."""
    nc = P.nc
    fp32 = mybir.dt.float32
    fp16 = mybir.dt.float16
    AF = mybir.ActivationFunctionType
    OP = mybir.AluOpType
    k = st["k"]
    sh = P.shared

    # ---------------- gather hop + 8-way sum ----------------
    g16 = P.smallp.tile([C, 16], fp32, tag="g16", name=f"g16_{k}")
    cc_ap = st["cc_out"].opt()
    nc.sync.dma_start(out=g16[:], in_=bass.AP(
        tensor=cc_ap.tensor, offset=cc_ap.offset,
        ap=[[2, C], [2 * C, N_CORES], [1, 2]]))
    t8 = P.smallp.tile([C, 8], fp32, tag="t8", name=f"t8_{k}")
    t4 = P.smallp.tile([C, 4], fp32, tag="t4", name=f"t4_{k}")
    g2 = P.smallp.tile([C, 2], fp32, tag="g2", name=f"g2_{k}")
    nc.vector.tensor_add(t8[:], g16[:, 0:8], g16[:, 8:16])
    nc.vector.tensor_add(t4[:], t8[:, 0:4], t8[:, 4:8])
    nc.vector.tensor_add(g2[:], t4[:, 0:2], t4[:, 2:4])

    # ---------------- BN epilogue: s = gamma*rsqrt/0.538, b = ... ----------
    vge = P.smallp.tile([C, 1], fp32, tag="vge", name=f"vge_{k}")
    gm2 = P.smallp.tile([C, 1], fp32, tag="gm2", name=f"gm2_{k}")
    nc.vector.tensor_mul(gm2[:], g2[:, 0:1], g2[:, 0:1])
    nc.vector.tensor_sub(vge[:], g2[:, 1:2], gm2[:])
    nc.vector.tensor_scalar_add(vge[:], vge[:], BN_EPS)
    rx = P.smallp.tile([C, 1], fp32, tag="rx", name=f"rx_{k}")
    nc.scalar.activation(out=rx[:], in_=vge[:], func=AF.Sqrt)
    nc.vector.reciprocal(out=rx[:], in_=rx[:])
    tX = P.smallp.tile([C, 1], fp32, tag="tX", name=f"tX_{k}")
    for _ in range(2):
        nc.vector.tensor_mul(tX[:], rx[:], rx[:])
        nc.vector.tensor_mul(tX[:], vge[:], tX[:])
        nc.vector.tensor_scalar(tX[:], tX[:], -0.5, 1.5, OP.mult, OP.add)
        nc.vector.tensor_mul(rx[:], rx[:], tX[:])

    s_q = P.smallp.tile([C, 1], fp32, tag="s_q", name=f"s_q_{k}")
    b_q = P.smallp.tile([C, 1], fp32, tag="b_q", name=f"b_q_{k}")
    ta = P.smallp.tile([C, 1], fp32, tag="ta", name=f"ta_{k}")
    tb = P.smallp.tile([C, 1], fp32, tag="tb", name=f"tb_{k}")
    nc.vector.tensor_mul(ta[:], st["gb"][:, 0:1], rx[:])
    nc.vector.tensor_scalar_mul(s_q[:], ta[:], 1.0 / HWGQ_STEP)
    nc.vector.tensor_mul(tb[:], g2[:, 0:1], ta[:])
    nc.vector.tensor_sub(tb[:], st["gb"][:, 1:2], tb[:])
    nc.vector.tensor_scalar_mul(b_q[:], tb[:], 1.0 / HWGQ_STEP)

    # ---------------- PE warmups (p-state ramp into the conv burst) --------
    if "nowarm" not in ablate:
        wu = P.smallp.tile([C, 128], fp32, tag="wu", name=f"wu_{k}")
        nc.vector.tensor_scalar_mul(wu[:], st["xA"][0][:, 0:128], g2[:, 0:1])
        ps_warm = P.psmallp.tile([C, 128], fp32, tag="psm", name=f"psw_{k}")
        for _ in range(6):
            nc.tensor.matmul(ps_warm[:], lhsT=sh["ones"][:], rhs=wu[:],
                             start=True, stop=True)

    # ---------------- per-image quantize + conv + drain + y ----------------
    wq, alpha = st["wq"], st["alpha"]
    a_t = sh["a_t"]
    out_sbs = [None] * IMG
    pss = [None] * IMG

    def quantize(i):
        u_sb = P.tmpp.tile([C, S], fp32, tag="u", name=f"u_{k}_{i}")
        # BN-apply chunks: ScalarE takes 3, Pool takes (1792:2688)
        nc.scalar.activation(out=u_sb[:, 0:896], in_=st["xA"][i][:, 0:896],
                             func=AF.Identity, bias=b_q[:], scale=s_q[:])
        nc.scalar.activation(out=u_sb[:, 896:SA], in_=st["xA"][i][:, 896:SA],
                             func=AF.Identity, bias=b_q[:], scale=s_q[:])
        nc.gpsimd.tensor_scalar(u_sb[:, SA:2688], st["xB"][i][:, 0:896],
                                s_q[:], b_q[:], OP.mult, OP.add)
        nc.scalar.activation(out=u_sb[:, 2688:S], in_=st["xB"][i][:, 896:SB],
                             func=AF.Identity, bias=b_q[:], scale=s_q[:])
        # clip (in place): DVE rows (0,16),(48,56); Pool rows (16,32),(32,48)
        for (r0, r1, eng) in ((0, 16, "D"), (16, 32, "P"),
                              (32, 48, "P"), (48, 56, "D")):
            lo, hi = r0 * HW, r1 * HW
            e = nc.vector if eng == "D" else nc.gpsimd
            e.tensor_scalar(u_sb[:, lo:hi], u_sb[:, lo:hi], 3.0, 0.0,
                            OP.min, OP.max)
        # round -> fp8 interior (DVE)
        for (r0, r1) in ((0, 16), (16, 32), (32, 48), (48, 56)):
            lo, hi = r0 * HW, r1 * HW
            nc.vector.tensor_scalar(a_t[i][:, r0 + 1:r1 + 1, 2:58],
                                    u_sb[:, lo:hi].rearrange(
                                        "p (h w) -> p h w", h=r1 - r0),
                                    MAGIC, MAGIC, OP.add, OP.subtract)
        out_sbs[i] = P.outp.tile([C, S], fp16, tag="o", name=f"o_{k}_{i}")
        pss[i] = [P.psump.tile([C, NFREE], fp32, tag=f"ps{c}",
                               name=f"ps_{k}_{i}_{c}") for c in range(NT)]

    def conv_chunk(i, cix, g):
        base = a_t[i][:]
        ps = pss[i]
        h0 = cix * R
        if g < 3:
            kw = g
            rhs = bass.AP(
                tensor=base.tensor,
                offset=base.offset + (h0 + 0) * PCW + (kw + 1),
                ap=[base.ap[0], [PCW, 2], [PCW, R], [1, HW]],
            )
            nc.tensor.matmul(ps[cix][:], lhsT=wq[:, 2 * kw: 2 * kw + 2, :],
                             rhs=rhs, start=(g == 0), stop=False,
                             perf_mode=mybir.MatmulPerfMode.DoubleRow)
        elif g == 3:
            # horizontal pair (kh=2, kw=0)&(kh=2, kw=1): slots 6,7
            rhs = bass.AP(
                tensor=base.tensor,
                offset=base.offset + (h0 + 2) * PCW + 1,
                ap=[base.ap[0], [1, 2], [PCW, R], [1, HW]],
            )
            nc.tensor.matmul(ps[cix][:], lhsT=wq[:, 6:8, :],
                             rhs=rhs, start=False, stop=False,
                             perf_mode=mybir.MatmulPerfMode.DoubleRow)
        else:
            rhs = a_t[i][:, h0 + 2: h0 + 2 + R, 3: 3 + HW]
            nc.tensor.matmul(ps[cix][:], lhsT=wq[:, 8, :], rhs=rhs,
                             start=False, stop=True)

    def drain_pair(i, pair, eng):
        ps = pss[i]
        for cix in pair:
            lo = cix * NFREE
            dst = out_sbs[i][:, lo:lo + NFREE]
            if eng == "S":
                nc.scalar.activation(out=dst, in_=ps[cix][:], func=AF.Identity,
                                     scale=alpha[:])
            else:
                nc.vector.tensor_scalar_mul(dst, ps[cix][:], alpha[:])

    PAIRS = ((0, 1), (2, 3), (4, 5), (6,))
    PAIR_ENG = {0: "S", 1: "S", 2: "S", 3: "D"}

    def conv_image(i):
        for pi, pair in enumerate(PAIRS):
            for g in range(5):
                for cix in pair:
                    conv_chunk(i, cix, g)
            drain_pair(i, pair, PAIR_ENG[pi])
            lo = pair[0] * NFREE
            hi = (pair[-1] + 1) * NFREE
            nc.sync.dma_start(out=P.y_d.ap()[i][:, lo:hi],
                              in_=out_sbs[i][:, lo:hi])

    # stagger: quantize(i) stays one image ahead of conv(i-1) so the
    # ScalarE/DVE queues never block the next image's act/clip behind the
    # previous image's PSUM drains.
    for i in range(IMG):
        if ld is not None:
            _emit_x_dmas(P, ld, i)
            if i == 0:
                nc.sync.dma_start(out=ld["w_sb"][:], in_=P.w_d.ap())
            if i == 1:
                _emit_gb_dma(P, ld)
        quantize(i)
        if ld is not None:
            _emit_stats(P, ld, i)
            if i == IMG - 1:
                _emit_payload(P, ld)
        if i >= 1:
            conv_image(i - 1)
    conv_image(IMG - 1)

    if ld is not None:
        # Pool order: collective first (it only waits on the payload DMA),
        # then the weight-quantize ops which depend on the late w DMA.
        _emit_collective(P, ld, ablate)
        _emit_weight_path(P, ld)


def _emit_cold_start(P, st, ablate):
    """First iteration's phase 1: loads + stats + payload + collective +
    weight path, plus one-time shared init (identity, ones, a_t borders)."""
    nc = P.nc
    fp32 = mybir.dt.float32
    fp8 = mybir.dt.float8e4
    sh = P.shared

    for i in range(IMG):
        _emit_x_dmas(P, st, i)
        _emit_stats(P, st, i)
    _emit_gb_dma(P, st)
    nc.sync.dma_start(out=st["w_sb"][:], in_=P.w_d.ap())
    _emit_payload(P, st)

    # one-time shared init
    sh["ident"] = P.smallp.tile([C, 128], fp32, tag="ident", name="ident")
    make_identity(nc, sh["ident"][:])
    sh["ones"] = P.smallp.tile([C, 128], fp32, tag="ones", name="ones")
    nc.vector.memset(sh["ones"][:], 1.0)
    sh["a_t"] = [P.apadp.tile([C, PR, PCW], fp8, tag=f"a{i}", name=f"a_t{i}")
                 for i in range(IMG)]
    for i in range(IMG):
        nc.gpsimd.memset(sh["a_t"][i][:, 0, :], 0.0)
        nc.gpsimd.memset(sh["a_t"][i][:, 57, :], 0.0)
        nc.gpsimd.memset(sh["a_t"][i][:, 1:57, 0:2], 0.0)
        nc.gpsimd.memset(sh["a_t"][i][:, 1:57, 58:64], 0.0)

    _emit_collective(P, st, ablate)
    _emit_weight_path(P, st)


def _build(n_iters=1, ablate=()):
    fp32 = mybir.dt.float32

    nc = bacc.Bacc("TRN2", target_bir_lowering=False, debug=False,
                   num_devices=N_CORES)

    x_d = nc.declare_dram_parameter("x", [IMG, C, S], fp32, isOutput=False)
    gamma_d = nc.declare_dram_parameter("gamma", [C], fp32, isOutput=False)
    beta_d = nc.declare_dram_parameter("beta", [C], fp32, isOutput=False)
    w_d = nc.declare_dram_parameter("weight", [C, 128 * 9], fp32, isOutput=False)
    # fp16 output: halves the y DMA stream; upconverted on host.
    y_d = nc.declare_dram_parameter("y", [IMG, C, S], mybir.dt.float16,
                                    isOutput=True)
    params = (x_d, gamma_d, beta_d, w_d, y_d)

    with tile.TileContext(nc) as tc:
        with (
            tc.tile_pool(name="xp", bufs=2) as xp,
            tc.tile_pool(name="apad", bufs=1) as apadp,
            tc.tile_pool(name="wp", bufs=1) as wp,
            tc.tile_pool(name="tmp", bufs=2) as tmpp,
            tc.tile_pool(name="outp", bufs=4) as outp,
            tc.tile_pool(name="small", bufs=1) as smallp,
            tc.tile_pool(name="psum", bufs=1, space="PSUM") as psump,
            tc.tile_pool(name="psmall", bufs=1, space="PSUM") as psmallp,
            tc.tile_pool(name="dram", bufs=2, space="DRAM") as dramp,
        ):
            pools = (xp, apadp, wp, tmpp, outp, smallp, psump, psmallp, dramp)
            P = _P(nc, pools, params, {})
            st = _alloc_state(P, 0)
            _emit_cold_start(P, st, ablate)
            for k in range(1, n_iters):
                ld = _alloc_state(P, k)
                _emit_block(P, st, ld, ablate)
                st = ld
            _emit_block(P, st, None, ablate)

    nc.finalize()
    return nc


def _get_nc(n_iters=1):
    key = ("nc", n_iters)
    if key not in _CACHE:
        _CACHE[key] = _build(n_iters)
    return _CACHE[key]


def make_in_maps(x, gamma, beta, weight):
    x = np.ascontiguousarray(np.asarray(x, np.float32)).reshape(N_CORES, IMG, C, S)
    w = np.ascontiguousarray(np.asarray(weight, np.float32)).reshape(C, 128 * 9)
    gamma = np.ascontiguousarray(np.asarray(gamma, np.float32))
    beta = np.ascontiguousarray(np.asarray(beta, np.float32))
    return [
        {"x": x[c], "gamma": gamma, "beta": beta, "weight": w}
        for c in range(N_CORES)
    ]


def kernel(x, gamma, beta, weight):
    import os
    from concourse.bass_utils import run_bass_kernel_spmd

    nc = _get_nc()
    in_maps = make_in_maps(x, gamma, beta, weight)
    core_ids = list(range(N_CORES))
    try:
        res = run_bass_kernel_spmd(nc, in_maps, core_ids)
    except ModuleNotFoundError:
        # BASS_TRACE set but no NTFF profile hook in this container
        os.environ["BASS_NEVER_TRACE"] = "1"
        res = run_bass_kernel_spmd(nc, in_maps, core_ids)
    out = np.stack([res.results[c]["y"] for c in range(N_CORES)], axis=0)
    return out.reshape(32, C, HW, HW).astype(np.float32)
